# revision 10
# baseline (speedup 1.0000x reference)
"""ErnieRna self-attention TRN2 Bass kernel (8-core SPMD).

Sharding: core i -> batch b = i//2, heads 6*(i%2) .. 6*(i%2)+5.
All attention math on-device; CPU only shards inputs / reassembles outputs
and provides constant tables (identity, ones, transposed weight layouts).

Per-core device structure:
  - projections q/k/v from hidden via PE (f32r), hidden transposed on PE
  - relative-position scores via the skew trick: QE = q @ E_rev^T windows,
    KE = k @ (E*s)^T windows, then diagonal SBUF->SBUF DMA gathers
    (negative-drift flat APs: step = row-1, verified exact on HW)
  - scores assembled in PSUM: QK + transpose(rel2T) + bias + mask, one DVE
    add folds in rel1; exp on ACT with fused row-sum accumulate
  - AV via PE transposes of W and v^T-style matmuls; final transpose back
"""

import contextlib
import numpy as np
import ml_dtypes

import concourse.bacc as bacc
import concourse.mybir as mybir
from concourse.ap import AP
from concourse.tile import TileContext
from concourse.bass_utils import run_bass_kernel_spmd

B, L, HID, H, D = 4, 1024, 768, 12, 64
MAX_POS = 1026
SCALING = D ** -0.5
HPC = 6               # heads per core
NCORES = 8
NH = HPC * D          # 384 output columns per core
EW = 2 * MAX_POS - 1  # 2051 dist_emb rows
WIN = 1152            # gather window width (row size of QE/KE tiles)

F32 = mybir.dt.float32
F32R = mybir.dt.float32r
BF16 = mybir.dt.bfloat16
AF = mybir.ActivationFunctionType
ALU = mybir.AluOpType

_CACHE = {}
LAST_EXEC_NS = None
TRACE = False


def _emit_projections(nc, tc, ctx, t, h_d, wqT_d, wkT_d, wvT_d):
    jw = ctx.enter_context(tc.tile_pool(name="projw", bufs=1))
    jl = ctx.enter_context(tc.tile_pool(name="projld", bufs=3))
    jps = ctx.enter_context(tc.tile_pool(name="projps", bufs=2, space="PSUM"))

    wq = [jw.tile([128, NH], F32R, tag=f"wq{cc}", name=f"wq{cc}") for cc in range(6)]
    wk = [jw.tile([128, NH], F32R, tag=f"wk{cc}", name=f"wk{cc}") for cc in range(6)]
    wv = [jw.tile([128, NH], F32R, tag=f"wv{cc}", name=f"wv{cc}") for cc in range(6)]
    hT = [jw.tile([128, L], F32R, tag=f"hT{cc}", name=f"hT{cc}") for cc in range(6)]
    for cc in range(6):
        nc.sync.dma_start(out=wq[cc][:], in_=wqT_d[cc * 128:(cc + 1) * 128, :])
        nc.sync.dma_start(out=wk[cc][:], in_=wkT_d[cc * 128:(cc + 1) * 128, :])
        nc.sync.dma_start(out=wv[cc][:], in_=wvT_d[cc * 128:(cc + 1) * 128, :])

    # hT[cc][:, lb*128:+128] = h[lb-block, cc-block]^T via PE
    for lb in range(8):
        hl = jl.tile([128, HID], F32, tag="hload", name="hl")
        nc.sync.dma_start(out=hl[:], in_=h_d[lb * 128:(lb + 1) * 128, :])
        for cc in range(6):
            tp = jps.tile([128, 128], F32, tag="tps", name="tp")
            nc.tensor.matmul(tp[:], hl[:, cc * 128:(cc + 1) * 128],
                             t["identf"][:], is_transpose=True)
            if cc % 2 == 0:
                nc.scalar.copy(out=hT[cc][:, lb * 128:(lb + 1) * 128], in_=tp[:])
            else:
                nc.vector.tensor_copy(hT[cc][:, lb * 128:(lb + 1) * 128], tp[:])

    # qT/kT: [d(2 heads)=128, L] per group g
    for g in range(3):
        for lc in range(2):
            ps = jps.tile([128, 512], F32, tag="qkps", name="ps", bufs=4)
            for cc in range(6):
                nc.tensor.matmul(ps[:], wq[cc][:, g * 128:(g + 1) * 128],
                                 hT[cc][:, lc * 512:(lc + 1) * 512],
                                 start=(cc == 0), stop=(cc == 5))
            nc.vector.tensor_scalar(
                out=t["qT"][g][:, lc * 512:(lc + 1) * 512], in0=ps[:],
                scalar1=float(SCALING), scalar2=t["bqc"][:, g:g + 1],
                op0=ALU.mult, op1=ALU.add)
            ps2 = jps.tile([128, 512], F32, tag="qkps", name="ps2", bufs=4)
            for cc in range(6):
                nc.tensor.matmul(ps2[:], wk[cc][:, g * 128:(g + 1) * 128],
                                 hT[cc][:, lc * 512:(lc + 1) * 512],
                                 start=(cc == 0), stop=(cc == 5))
            nc.vector.tensor_scalar(
                out=t["kT"][g][:, lc * 512:(lc + 1) * 512], in0=ps2[:],
                scalar1=t["bkc"][:, g:g + 1], scalar2=None, op0=ALU.add)

    # v[rb] = h-block @ WvT + bv  -> [r=128, 384]
    for rb in range(8):
        ps = jps.tile([128, NH], F32, tag="vps", name="vps")
        for cc in range(6):
            nc.tensor.matmul(ps[:], hT[cc][:, rb * 128:(rb + 1) * 128],
                             wv[cc][:], start=(cc == 0), stop=False)
        nc.tensor.matmul(ps[:], t["onesr"][:], t["bvr"][:], start=False, stop=True)
        nc.scalar.copy(out=t["vv"][rb][:], in_=ps[:])


def _emit_head(nc, tc, t, p, hh, bias_d, sc_d, wt_d):
    g = hh // 2
    if hh % 2 == 0:
        qh = t["qT"][g][0:64, :]
        kh = t["kT"][g][0:64, :]
    else:
        # PE requires matching lhsT/rhs partition bases; move the upper-half
        # head rows down to base partition 0 via SBUF->SBUF DMA.
        qh = p["hsp"].tile([64, L], F32R, tag="qh_s", name="qh_s")
        kh = p["hsp"].tile([64, L], F32R, tag="kh_s", name="kh_s")
        nc.sync.dma_start(out=qh[:], in_=t["qT"][g][64:128, :])
        nc.sync.dma_start(out=kh[:], in_=t["kT"][g][64:128, :])

    # B: KE windows + rel2T gathers (per r-block)
    rel2_t = []
    for rb in range(8):
        a2 = 898 - rb * 128
        ke_t = p["qep"].tile([128, WIN], F32, tag="ke", name="ke_t")
        for ci, (c0, cw) in enumerate(((0, 512), (512, 512), (1024, 128))):
            ps = p["qkps"].tile([128, 512], F32, tag="qkeps", name="keps")
            nc.tensor.matmul(ps[:, :cw], kh[:, rb * 128:(rb + 1) * 128],
                             t["etfs"][:, a2 + c0:a2 + c0 + cw])
            if ci % 2 == 0:
                nc.vector.tensor_copy(ke_t[:, c0:c0 + cw], ps[:, :cw])
            else:
                nc.scalar.copy(out=ke_t[:, c0:c0 + cw], in_=ps[:, :cw])
        r2 = p["relp"].tile([128, L], F32, tag=f"r2t{rb}", name=f"r2t{rb}")
        nc.scalar.dma_start(
            out=r2[:], in_=AP(ke_t.tensor, 127, [[WIN - 1, 128], [1, L]]))
        rel2_t.append(r2)

    # A: QE windows + rel1 gathers (per l-block)
    rel1_t = []
    for lb in range(8):
        a = 897 - lb * 128
        qe_t = p["qep"].tile([128, WIN], BF16, tag="qe", name="qe_t")
        for ci, (c0, cw) in enumerate(((0, 512), (512, 512), (1024, 128))):
            ps = p["qkps"].tile([128, 512], F32, tag="qkeps", name="qeps")
            nc.tensor.matmul(ps[:, :cw], qh[:, lb * 128:(lb + 1) * 128],
                             t["etrev"][:, a + c0:a + c0 + cw])
            if ci % 2 == 0:
                nc.scalar.copy(out=qe_t[:, c0:c0 + cw], in_=ps[:, :cw])
            else:
                nc.vector.tensor_copy(qe_t[:, c0:c0 + cw], ps[:, :cw])
        r1 = p["r1p"].tile([128, L], BF16, tag="r1", name="r1")
        nc.sync.dma_start(
            out=r1[:], in_=AP(qe_t.tensor, 128, [[WIN - 1, 128], [1, L]]))
        rel1_t.append(r1)

    # C+D interleaved per half of l-blocks so only 4 W tiles live at once.
    for lcg in range(2):
        # C: scores assembly, exp, weights (per l-block in this half)
        w_t = []
        for lb in range(lcg * 4, lcg * 4 + 4):
            bias_t = p["bp"].tile([128, L], F32R, tag="bias", name="bias_t")
            nc.sync.dma_start(out=bias_t[:],
                              in_=bias_d[hh, lb * 128:(lb + 1) * 128, :])
            s_t = p["sep"].tile([128, L], F32, tag="s", name="s_t")
            for rc in range(2):
                sp = p["sps"].tile([128, 512], F32, tag="sp", name="sp")
                nc.tensor.matmul(sp[:], qh[:, lb * 128:(lb + 1) * 128],
                                 kh[:, rc * 512:(rc + 1) * 512],
                                 start=True, stop=False)
                for i in range(4):
                    rb = rc * 4 + i
                    nc.tensor.matmul(sp[:, i * 128:(i + 1) * 128],
                                     rel2_t[rb][:, lb * 128:(lb + 1) * 128],
                                     t["identf"][:], is_transpose=True,
                                     start=False, stop=False)
                nc.tensor.matmul(sp[:], t["identr"][:],
                                 bias_t[:, rc * 512:(rc + 1) * 512],
                                 start=False, stop=False)
                nc.tensor.matmul(sp[:], t["onesr"][:],
                                 t["mask_sb"][:, rc * 512:(rc + 1) * 512],
                                 start=False, stop=True)
                nc.vector.tensor_add(s_t[:, rc * 512:(rc + 1) * 512], sp[:],
                                     rel1_t[lb][:, rc * 512:(rc + 1) * 512])
            nc.sync.dma_start(out=sc_d[hh, lb * 128:(lb + 1) * 128, :], in_=s_t[:])
            e_t = p["sep"].tile([128, L], F32, tag="e", name="e_t")
            z_t = p["zp"].tile([128, 1], F32, tag="z", name="z_t")
            nc.scalar.activation(e_t[:], s_t[:], AF.Exp, accum_out=z_t[:])
            rz_t = p["zp"].tile([128, 1], F32, tag="rz", name="rz_t")
            nc.vector.reciprocal(rz_t[:], z_t[:])
            w = p["wp"].tile([128, L], F32, tag=f"w{lb % 4}", name=f"w{lb % 4}")
            nc.vector.tensor_scalar_mul(w[:], e_t[:], rz_t[:])
            nc.sync.dma_start(out=wt_d[hh, lb * 128:(lb + 1) * 128, :], in_=w[:])
            w_t.append(w)

        # D: AV for this l-half. The av PSUM accumulation groups must not
        # interleave with other PE work, so pre-stage 4 transposed-W tiles,
        # then run a contiguous 4-matmul group; halves merged via DVE add.
        avp = p["avps"].tile([64, 512], F32, tag="av", name="avp")
        avh = p["avsp"].tile([64, 512], F32, tag="avh", name="avh")
        for half in range(2):
            wts = []
            for i in range(4):
                rb = half * 4 + i
                wtps_t = p["wtps"].tile([128, 512], F32, tag="wtp", name="wtps_t")
                for j in range(4):
                    nc.tensor.matmul(wtps_t[:, j * 128:(j + 1) * 128],
                                     w_t[j][:, rb * 128:(rb + 1) * 128],
                                     t["identf"][:], is_transpose=True,
                                     start=(j == 0), stop=(j == 3))
                wt_t = p["wtp"].tile([128, 512], F32R, tag="wt", name="wt_t", bufs=6)
                nc.scalar.copy(out=wt_t[:], in_=wtps_t[:])
                wts.append((rb, wt_t))
            for i, (rb, wt_t) in enumerate(wts):
                nc.tensor.matmul(avp[:], t["vv"][rb][:, hh * 64:hh * 64 + 64],
                                 wt_t[:], start=(i == 0), stop=(i == 3))
            if half == 0:
                nc.scalar.copy(out=avh[:], in_=avp[:])
        av_t = p["avsp"].tile([64, 512], F32, tag="av_sb", name="av_t")
        nc.vector.tensor_add(av_t[:], avp[:], avh[:])

        # E: transpose AV result back to [l, d], place into attn_sb
        for j in range(4):
            lb = lcg * 4 + j
            ot = p["wtps"].tile([128, 64], F32, tag="wtp", name="ot")
            nc.tensor.matmul(ot[:], av_t[:, j * 128:(j + 1) * 128],
                             t["identf"][:64, :64], is_transpose=True)
            nc.scalar.copy(out=t["attn_sb"][lb][:, hh * 64:hh * 64 + 64], in_=ot[:])


def _build():
    nc = bacc.Bacc("TRN2", target_bir_lowering=False)

    h_d = nc.dram_tensor("h", [L, HID], F32, kind="ExternalInput")
    wqT_d = nc.dram_tensor("wqT", [HID, NH], F32R, kind="ExternalInput")
    wkT_d = nc.dram_tensor("wkT", [HID, NH], F32R, kind="ExternalInput")
    wvT_d = nc.dram_tensor("wvT", [HID, NH], F32R, kind="ExternalInput")
    bq_d = nc.dram_tensor("bqc", [128, 3], F32, kind="ExternalInput")
    bk_d = nc.dram_tensor("bkc", [128, 3], F32, kind="ExternalInput")
    bv_d = nc.dram_tensor("bvr", [1, NH], F32R, kind="ExternalInput")
    etrev_d = nc.dram_tensor("eTrev", [D, EW], F32R, kind="ExternalInput")
    etfs_d = nc.dram_tensor("eTfs", [D, EW], F32R, kind="ExternalInput")
    bias_d = nc.dram_tensor("bias6", [HPC, L, L], F32R, kind="ExternalInput")
    mask_d = nc.dram_tensor("mask", [1, L], F32R, kind="ExternalInput")
    identf_d = nc.dram_tensor("identf", [128, 128], F32, kind="ExternalInput")
    identr_d = nc.dram_tensor("identr", [128, 128], F32R, kind="ExternalInput")
    onesr_d = nc.dram_tensor("onesr", [1, 128], F32R, kind="ExternalInput")

    attn_d = nc.dram_tensor("attn", [L, NH], F32, kind="ExternalOutput")
    sc_d = nc.dram_tensor("scores6", [HPC, L, L], F32, kind="ExternalOutput")
    wt_d = nc.dram_tensor("weights6", [HPC, L, L], F32, kind="ExternalOutput")

    with TileContext(nc) as tc, contextlib.ExitStack() as ctx:
        cp = ctx.enter_context(tc.tile_pool(name="consts", bufs=1))
        pp = ctx.enter_context(tc.tile_pool(name="persist", bufs=1))

        t = {}
        t["identf"] = cp.tile([128, 128], F32, tag="identf", name="identf")
        t["identr"] = cp.tile([128, 128], F32R, tag="identr", name="identr")
        t["onesr"] = cp.tile([1, 128], F32R, tag="onesr", name="onesr")
        t["mask_sb"] = cp.tile([1, L], F32R, tag="mask", name="mask_sb")
        t["bqc"] = cp.tile([128, 3], F32, tag="bqc", name="bqc")
        t["bkc"] = cp.tile([128, 3], F32, tag="bkc", name="bkc")
        t["bvr"] = cp.tile([1, NH], F32R, tag="bvr", name="bvr")
        t["etrev"] = cp.tile([D, EW], F32R, tag="etrev", name="etrev")
        t["etfs"] = cp.tile([D, EW], F32R, tag="etfs", name="etfs")
        for nm, d_ in (("identf", identf_d), ("identr", identr_d),
                       ("onesr", onesr_d), ("mask_sb", mask_d),
                       ("bqc", bq_d), ("bkc", bk_d), ("bvr", bv_d),
                       ("etrev", etrev_d), ("etfs", etfs_d)):
            nc.sync.dma_start(out=t[nm][:], in_=d_[:])

        t["qT"] = [pp.tile([128, L], F32R, tag=f"qT{g}", name=f"qT{g}") for g in range(3)]
        t["kT"] = [pp.tile([128, L], F32R, tag=f"kT{g}", name=f"kT{g}") for g in range(3)]
        t["vv"] = [pp.tile([128, NH], F32R, tag=f"v{rb}", name=f"v{rb}") for rb in range(8)]
        t["attn_sb"] = [pp.tile([128, NH], F32, tag=f"attn{lb}", name=f"attn{lb}")
                        for lb in range(8)]

        with contextlib.ExitStack() as jctx:
            _emit_projections(nc, tc, jctx, t, h_d, wqT_d, wkT_d, wvT_d)

        p = {}
        p["qep"] = ctx.enter_context(tc.tile_pool(name="qe", bufs=2))
        p["relp"] = ctx.enter_context(tc.tile_pool(name="rel", bufs=1))
        p["r1p"] = ctx.enter_context(tc.tile_pool(name="r1", bufs=8))
        p["bp"] = ctx.enter_context(tc.tile_pool(name="biasld", bufs=2))
        p["sep"] = ctx.enter_context(tc.tile_pool(name="sE", bufs=2))
        p["wp"] = ctx.enter_context(tc.tile_pool(name="wp", bufs=1))
        p["wtp"] = ctx.enter_context(tc.tile_pool(name="wtp", bufs=6))
        p["avsp"] = ctx.enter_context(tc.tile_pool(name="avp", bufs=2))
        p["zp"] = ctx.enter_context(tc.tile_pool(name="zp", bufs=4))
        p["hsp"] = ctx.enter_context(tc.tile_pool(name="hsp", bufs=1))
        p["qkps"] = ctx.enter_context(tc.tile_pool(name="qekeps", bufs=2, space="PSUM"))
        p["sps"] = ctx.enter_context(tc.tile_pool(name="sps", bufs=2, space="PSUM"))
        p["wtps"] = ctx.enter_context(tc.tile_pool(name="wtps", bufs=2, space="PSUM"))
        p["avps"] = ctx.enter_context(tc.tile_pool(name="avps", bufs=2, space="PSUM"))

        for hh in range(HPC):
            _emit_head(nc, tc, t, p, hh, bias_d, sc_d, wt_d)

        for lb in range(8):
            nc.sync.dma_start(out=attn_d[lb * 128:(lb + 1) * 128, :],
                              in_=t["attn_sb"][lb][:])

    nc.compile()
    return nc


def _shard_inputs(hidden_states, attention_mask, attention_bias,
                  Wq, bq, Wk, bk, Wv, bv, dist_emb):
    eye = np.eye(128, dtype=np.float32)
    eTrev = np.ascontiguousarray(dist_emb[::-1].T)            # [64, 2051]
    eTfs = np.ascontiguousarray(dist_emb.T) * np.float32(SCALING)
    in_maps = []
    for core in range(NCORES):
        b, hs = core // 2, 6 * (core % 2)
        sl = slice(hs * D, hs * D + NH)
        in_maps.append({
            "h": np.ascontiguousarray(hidden_states[b]),
            "wqT": np.ascontiguousarray(Wq[sl].T),
            "wkT": np.ascontiguousarray(Wk[sl].T),
            "wvT": np.ascontiguousarray(Wv[sl].T),
            "bqc": np.ascontiguousarray(bq[sl].reshape(3, 128).T),
            "bkc": np.ascontiguousarray(bk[sl].reshape(3, 128).T),
            "bvr": np.ascontiguousarray(bv[sl].reshape(1, NH)),
            "eTrev": eTrev,
            "eTfs": eTfs,
            "bias6": np.ascontiguousarray(attention_bias[b, hs:hs + HPC]),
            "mask": np.ascontiguousarray(
                np.broadcast_to(attention_mask[b, 0, 0], (L,)).reshape(1, L)),
            "identf": eye,
            "identr": eye,
            "onesr": np.ones((1, 128), dtype=np.float32),
        })
    return in_maps


def kernel(hidden_states, attention_mask, attention_bias,
           Wq, bq, Wk, bk, Wv, bv, dist_emb):
    global LAST_EXEC_NS
    if "nc" not in _CACHE:
        _CACHE["nc"] = _build()
    nc = _CACHE["nc"]
    in_maps = _shard_inputs(
        np.asarray(hidden_states, dtype=np.float32),
        np.asarray(attention_mask, dtype=np.float32),
        np.asarray(attention_bias, dtype=np.float32),
        np.asarray(Wq, dtype=np.float32), np.asarray(bq, dtype=np.float32),
        np.asarray(Wk, dtype=np.float32), np.asarray(bk, dtype=np.float32),
        np.asarray(Wv, dtype=np.float32), np.asarray(bv, dtype=np.float32),
        np.asarray(dist_emb, dtype=np.float32))

    res = run_bass_kernel_spmd(nc, in_maps, list(range(NCORES)), trace=TRACE)
    LAST_EXEC_NS = res.exec_time_ns

    attn = np.zeros((B, L, HID), np.float32)
    scores = np.zeros((B, H, L, L), np.float32)
    weights = np.zeros((B, H, L, L), np.float32)
    for core in range(NCORES):
        b, hs = core // 2, 6 * (core % 2)
        r = res.results[core]
        attn[b][:, hs * D:hs * D + NH] = r["attn"]
        scores[b, hs:hs + HPC] = r["scores6"]
        weights[b, hs:hs + HPC] = r["weights6"]
    return attn, scores, weights
